# revision 10
# baseline (speedup 1.0000x reference)
"""BevFormer block (temporal attn + spatial cross-attn + FFN, 3 LayerNorms)
as a Bass/Tile kernel for 8 Trainium2 NeuronCores.

Sharding: data-parallel over batch (2) x query-row chunks (4) -> 8 cores.
Each core computes the full block for its 625 query rows; K/V projections
(over the full 2500 keys of its batch) are duplicated across the 4 cores
sharing a batch.

Device layout is feature-major throughout ([E on partitions, tokens on the
free axis]); inputs are pre-transposed on the host so the device never
transposes. Attention: per-head QK matmuls (K=D=32) with head data
consolidated to partitions 0-31, exp on ScalarE straight out of PSUM, and
col-tiled PV matmuls with a ones-column producing softmax denominators.
Per-token scalars (softmax denominators, LN stats) are broadcast across
partitions with a DRAM round-trip DMA.
"""

import numpy as np
import ml_dtypes
from contextlib import ExitStack

B, Q, P, K, E, H = 2, 2500, 2500, 2500, 256, 8
D = E // H
FF = 2 * E
EPS = 1e-5
NQ = Q // 4          # 625 query rows per core
NK = K               # 2500 keys
NKT = 125            # nk tile (partition dim of S^T)
NT = NK // NKT       # 20
CHUNKS = [(0, 256), (256, 256), (512, 113)]   # nq chunks for attention core
C2 = [(0, 320), (320, 305)]                   # nq chunks for projections

_CACHE = {}


def _build_nc():
    import concourse.bass as bass
    import concourse.bacc as bacc
    import concourse.tile as tile
    from concourse import mybir

    F32 = mybir.dt.float32
    BF16 = mybir.dt.bfloat16
    AF = mybir.ActivationFunctionType
    ALU = mybir.AluOpType

    nc = bacc.Bacc("TRN2", target_bir_lowering=False, debug=False)

    dr = {}

    def din(name, shape, dt=BF16):
        dr[name] = nc.dram_tensor(name, shape, dt, kind="ExternalInput").ap()

    din("qT_bf", [E, NQ])
    din("qT_f32", [E, NQ], F32)
    din("prevT", [E, NK])
    din("imgT", [E, NK])
    for w in ("t_wq", "t_wk", "t_wv", "s_wq", "s_wk", "s_wv", "s_wo"):
        din(w, [E, E])
    din("ff_w1", [E, FF])
    din("ff_w2", [FF, E])
    for v, t in (("s_bo", 2), ("ff_b1", 4), ("ff_b2", 2), ("n1_g", 2),
                 ("n1_b", 2), ("n2_g", 2), ("n2_b", 2), ("n3_g", 2),
                 ("n3_b", 2)):
        din(v, [t, 128], F32)
    out_d = nc.dram_tensor("outT", [E, NQ], F32, kind="ExternalOutput").ap()

    with ExitStack() as ctx:
        tc = ctx.enter_context(tile.TileContext(nc))
        consts = ctx.enter_context(tc.tile_pool(name="consts", bufs=1))
        big = ctx.enter_context(tc.tile_pool(name="big", bufs=1))
        pexp = ctx.enter_context(tc.tile_pool(name="pexp", bufs=3))
        pp = ctx.enter_context(tc.tile_pool(name="pp", bufs=2, space="PSUM"))
        qkp = ctx.enter_context(tc.tile_pool(name="qkp", bufs=2, space="PSUM"))
        opn = ctx.enter_context(tc.tile_pool(name="opn", bufs=1, space="PSUM"))
        dramp = ctx.enter_context(tc.tile_pool(name="dramp", bufs=1,
                                               space="DRAM"))

        # ---- load constants / inputs ----
        def load_w(name, kt, m):
            t = consts.tile([128, kt, m], BF16, tag=name)
            nc.sync.dma_start(out=t, in_=dr[name].rearrange(
                "(kt p) m -> p kt m", p=128))
            return t

        wts = {w: load_w(w, 2, E) for w in
               ("t_wq", "t_wk", "t_wv", "s_wq", "s_wk", "s_wv", "s_wo")}
        ffw1 = load_w("ff_w1", 2, FF)
        ffw2 = load_w("ff_w2", 4, E)

        def load_vec(name, t):
            v = consts.tile([128, t], F32, tag=name)
            nc.sync.dma_start(out=v, in_=dr[name].rearrange("t p -> p t"))
            return v

        vecs = {n: load_vec(n, t) for n, t in
                (("s_bo", 2), ("ff_b1", 4), ("ff_b2", 2), ("n1_g", 2),
                 ("n1_b", 2), ("n2_g", 2), ("n2_b", 2), ("n3_g", 2),
                 ("n3_b", 2))}

        qT_bf = consts.tile([128, 2, NQ], BF16, tag="qT_bf")
        nc.sync.dma_start(out=qT_bf, in_=dr["qT_bf"].rearrange(
            "(kt p) n -> p kt n", p=128))
        # qT_f32 shares a slot with outT (dead before LN3 output is written)
        qT_f32 = big.tile([128, 2, NQ], F32, tag="qf32_out")
        nc.sync.dma_start(out=qT_f32, in_=dr["qT_f32"].rearrange(
            "(kt p) n -> p kt n", p=128))

        ones128 = consts.tile([128, 128], BF16, tag="ones128")
        nc.vector.memset(ones128, 1.0)
        eps_t = consts.tile([1, 1], F32, tag="eps")
        nc.vector.memset(eps_t, EPS)

        # v tile: [NKT, NT, 8, 64]; even heads: data cols 0:32, ones col 32;
        # odd heads: ones col 0, data cols 32:64. Init once.
        v_sb = big.tile([NKT, NT, 8, 64], BF16, tag="v")
        nc.vector.memset(v_sb, 0.0)
        for h in range(8):
            c1 = 32 if h % 2 == 0 else 0
            nc.vector.memset(v_sb[:, :, h, c1:c1 + 1], 1.0)

        def proj(w_sb, rhs_sb, out_sb, chunks, nkt=2, psl=128):
            # out[:, et, :] (bf16) = w[:, :, 128et:+128].T @ rhs  over kt
            net = out_sb.shape[1]
            for et in range(net):
                for (off, cs) in chunks:
                    ps = pp.tile([128, 512], mybir.dt.float32, tag="proj")
                    for kt in range(nkt):
                        nc.tensor.matmul(
                            ps[:psl, :cs],
                            lhsT=w_sb[:, kt, 128 * et:128 * et + 128][:, :psl],
                            rhs=rhs_sb[:, kt, off:off + cs],
                            start=(kt == 0), stop=(kt == nkt - 1))
                    nc.vector.tensor_copy(out=out_sb[:psl, et, off:off + cs],
                                          in_=ps[:psl, :cs])

        KCH = [(500 * i, 500) for i in range(5)]

        def attention(xT_bf, kvT, wq, wk, wv, escale, O_u, d_r):
            # K/V/Q projections (E-major), then consolidate heads to
            # partitions 0-31 via SBUF->SBUF DMA (row group 0 for QK).
            kT = big.tile([128, 2, NK], BF16, tag="kT")
            proj(wk, kvT, kT, KCH)
            kT_hm = big.tile([32, 8, NK], BF16, tag="kT_hm")
            for h in range(8):
                nc.sync.dma_start(out=kT_hm[:, h, :],
                                  in_=kT[32 * (h % 4):32 * (h % 4) + 32,
                                         h // 4, :])
            for t in range(NT):
                ps = pp.tile([128, 512], F32, tag="proj")
                for kt in range(2):
                    nc.tensor.matmul(
                        ps[:NKT, :E],
                        lhsT=kvT[:, kt, NKT * t:NKT * (t + 1)],
                        rhs=wv[:, kt, :],
                        start=(kt == 0), stop=(kt == 1))
                vp = ps[:NKT, :E].rearrange("p (h2 two d) -> p h2 two d",
                                            two=2, d=32)
                vd = v_sb[:, t].rearrange("p (h2 two) c -> p h2 two c", two=2)
                nc.vector.tensor_copy(out=vd[:, :, 0, 0:32],
                                      in_=vp[:, :, 0, :])
                nc.vector.tensor_copy(out=vd[:, :, 1, 32:64],
                                      in_=vp[:, :, 1, :])
            qTt = big.tile([128, 2, NQ], BF16, tag="qTt")
            proj(wq, xT_bf, qTt, C2)
            qT_hm = big.tile([32, 8, NQ], BF16, tag="qT_hm")
            for h in range(8):
                nc.sync.dma_start(out=qT_hm[:, h, :],
                                  in_=qTt[32 * (h % 4):32 * (h % 4) + 32,
                                          h // 4, :])

            d_all = big.tile([128, 2, NQ], F32, tag="d_a")
            nc.vector.memset(d_all, 1.0)

            for (off, cs) in CHUNKS:
                for w in range(2):
                    O_ps = opn.tile([128, 2, 512], F32)
                    for t in range(NT):
                        S_ps = qkp.tile([128, 4, 256], F32)
                        for j in range(4):
                            nc.tensor.matmul(
                                S_ps[:NKT, j, :cs],
                                lhsT=kT_hm[:, 4 * w + j,
                                           NKT * t:NKT * (t + 1)],
                                rhs=qT_hm[:, 4 * w + j, off:off + cs],
                                start=True, stop=True)
                        Pt = pexp.tile([NKT, 4, 256], BF16, tag="P")
                        nc.scalar.activation(Pt[:, :, :cs],
                                             S_ps[:NKT, :, :cs],
                                             AF.Exp, scale=escale)
                        st, sp = (t == 0), (t == NT - 1)
                        nc.tensor.matmul(
                            O_ps[0:33, 0, :cs],
                            lhsT=v_sb[:, t, 4 * w + 0, 0:33],
                            rhs=Pt[:, 0, :cs], start=st, stop=sp,
                            skip_group_check=True,
                            tile_position=(0, 0))
                        nc.tensor.matmul(
                            O_ps[64:97, 0, :cs],
                            lhsT=v_sb[:, t, 4 * w + 2, 0:33],
                            rhs=Pt[:, 2, :cs], start=st, stop=sp,
                            skip_group_check=True,
                            tile_position=(0, 64))
                        nc.tensor.matmul(
                            O_ps[0:64, 1, :cs],
                            lhsT=v_sb[:, t, 4 * w + 1, 0:64],
                            rhs=Pt[:, 1, :cs], start=st, stop=sp,
                            skip_group_check=True,
                            tile_position=(0, 0))
                        nc.tensor.matmul(
                            O_ps[64:128, 1, :cs],
                            lhsT=v_sb[:, t, 4 * w + 3, 0:64],
                            rhs=Pt[:, 3, :cs], start=st, stop=sp,
                            skip_group_check=True,
                            tile_position=(0, 64))
                    # drain: heads -> E rows, denominators -> d_all
                    wsl = slice(off, off + cs)
                    nc.vector.tensor_copy(out=O_u[0:32, w, wsl],
                                          in_=O_ps[0:32, 0, :cs])
                    nc.vector.tensor_copy(out=O_u[64:96, w, wsl],
                                          in_=O_ps[64:96, 0, :cs])
                    nc.vector.tensor_copy(out=O_u[32:64, w, wsl],
                                          in_=O_ps[32:64, 1, :cs])
                    nc.vector.tensor_copy(out=O_u[96:128, w, wsl],
                                          in_=O_ps[96:128, 1, :cs])
                    nc.vector.tensor_copy(out=d_all[32:33, w, wsl],
                                          in_=O_ps[32:33, 0, :cs])
                    nc.vector.tensor_copy(out=d_all[96:97, w, wsl],
                                          in_=O_ps[96:97, 0, :cs])
                    nc.vector.tensor_copy(out=d_all[0:1, w, wsl],
                                          in_=O_ps[0:1, 1, :cs])
                    nc.vector.tensor_copy(out=d_all[64:65, w, wsl],
                                          in_=O_ps[64:65, 1, :cs])
            # 1/d via exp(-ln(d)) (stays in the exp table set)
            t_ln = big.tile([128, 2, NQ], F32, tag="d_b")
            nc.scalar.activation(t_ln, d_all, AF.Ln)
            nc.scalar.activation(d_r, t_ln, AF.Exp, scale=-1.0)

        def make_dbc(d_r):
            scr = dramp.tile([4, 2, NQ], F32, tag="dscr")
            dbc = big.tile([128, 2, NQ], F32, tag="d_b")
            for i, (dst0, srow) in enumerate(((0, 32), (32, 0), (64, 96),
                                              (96, 64))):
                nc.sync.dma_start(out=scr[i:i + 1], in_=d_r[srow:srow + 1])
                row = scr[i:i + 1]
                bc = bass.AP(tensor=row.tensor, offset=row.offset,
                             ap=[[0, 32]] + list(row.ap)[1:])
                nc.sync.dma_start(out=dbc[dst0:dst0 + 32], in_=bc)
            return dbc

        def bcast_row(src_row, out_bc, scr):
            # broadcast [1, n] SBUF row -> [128, n] via DRAM round-trip
            n = src_row.free_size()
            nc.sync.dma_start(out=scr[:, :n], in_=src_row)
            row = scr[:, :n]
            bc = bass.AP(tensor=row.tensor, offset=row.offset,
                         ap=[[0, 128]] + list(row.ap)[1:])
            nc.sync.dma_start(out=out_bc, in_=bc)

        def layernorm(u_f32, g_sb, b_sb, x_bf, x_f32):
            u_bf = big.tile([128, 2, NQ], BF16, tag="ln_ubf")
            nc.vector.tensor_copy(out=u_bf, in_=u_f32)
            usq = big.tile([128, 2, NQ], BF16, tag="ln_usq")
            nc.vector.tensor_tensor(out=usq, in0=u_bf, in1=u_bf, op=ALU.mult)
            lv = big.tile([1, 8, NQ], F32, tag="ln_vec")
            for (off, cs) in C2:
                st1 = pp.tile([1, 512], F32, tag="proj")
                st2 = pp.tile([1, 512], F32, tag="proj")
                for kt in range(2):
                    nc.tensor.matmul(st1[:, :cs], lhsT=ones128[:, 0:1],
                                     rhs=u_bf[:, kt, off:off + cs],
                                     start=(kt == 0), stop=(kt == 1),
                                     tile_position=(0, 0))
                for kt in range(2):
                    nc.tensor.matmul(st2[:, :cs], lhsT=ones128[:, 0:1],
                                     rhs=usq[:, kt, off:off + cs],
                                     start=(kt == 0), stop=(kt == 1),
                                     tile_position=(0, 0))
                nc.vector.tensor_scalar(out=lv[:, 0, off:off + cs],
                                        in0=st1[:, :cs], scalar1=1.0 / E,
                                        scalar2=None, op0=ALU.mult)
                nc.vector.tensor_scalar(out=lv[:, 1, off:off + cs],
                                        in0=st2[:, :cs], scalar1=1.0 / E,
                                        scalar2=None, op0=ALU.mult)
            mu, msq = lv[:, 0, :], lv[:, 1, :]
            nc.vector.tensor_tensor(out=lv[:, 2, :], in0=mu, in1=mu,
                                    op=ALU.mult)
            nc.vector.tensor_tensor(out=lv[:, 3, :], in0=msq, in1=lv[:, 2, :],
                                    op=ALU.subtract)
            nc.scalar.activation(lv[:, 4, :], lv[:, 3, :], AF.Ln, bias=eps_t)
            nc.scalar.activation(lv[:, 5, :], lv[:, 4, :], AF.Exp, scale=-0.5)
            nc.vector.tensor_tensor(out=lv[:, 6, :], in0=mu, in1=lv[:, 5, :],
                                    op=ALU.mult)
            rb = big.tile([128, NQ], F32, tag="ln_rb")
            murb = big.tile([128, NQ], F32, tag="ln_murb")
            scr2 = dramp.tile([2, NQ], F32, tag="lnscr")
            bcast_row(lv[:, 5, :], rb, scr2[0:1])
            bcast_row(lv[:, 6, :], murb, scr2[1:2])
            for et in range(2):
                t1 = big.tile([128, NQ], F32, tag="ln_t1")
                t2 = big.tile([128, NQ], F32, tag="ln_t2")
                nc.vector.tensor_tensor(out=t1, in0=u_f32[:, et, :], in1=rb,
                                        op=ALU.mult)
                nc.vector.tensor_tensor(out=t2, in0=t1, in1=murb,
                                        op=ALU.subtract)
                dst = x_f32 if x_f32 is not None else x_bf
                nc.vector.tensor_scalar(out=dst[:, et, :], in0=t2,
                                        scalar1=g_sb[:, et:et + 1],
                                        scalar2=b_sb[:, et:et + 1],
                                        op0=ALU.mult, op1=ALU.add)
            if x_f32 is not None and x_bf is not None:
                nc.vector.tensor_copy(out=x_bf, in_=x_f32)

        # kvT slot shared by prevT and imgT (prevT dead after temporal projs)
        prevT = big.tile([128, 2, NK], BF16, tag="kvT")
        nc.sync.dma_start(out=prevT, in_=dr["prevT"].rearrange(
            "(kt p) n -> p kt n", p=128))

        # ================= temporal attention =================
        O1u = big.tile([128, 2, NQ], BF16, tag="Ou")
        d_r1 = big.tile([128, 2, NQ], F32, tag="d_a2")
        attention(qT_bf, prevT, wts["t_wq"], wts["t_wk"], wts["t_wv"],
                  1.0 / (float(E) ** 0.5), O1u, d_r1)
        dbc1 = make_dbc(d_r1)
        u1 = big.tile([128, 2, NQ], F32, tag="u")
        for et in range(2):
            tt = big.tile([128, NQ], F32, tag="res_t")
            nc.vector.tensor_tensor(out=tt, in0=O1u[:, et, :],
                                    in1=dbc1[:, et, :], op=ALU.mult)
            nc.vector.tensor_tensor(out=u1[:, et, :], in0=tt,
                                    in1=qT_f32[:, et, :], op=ALU.add)
        x1_bf = big.tile([128, 2, NQ], BF16, tag="x1_bf")
        x1_f32 = big.tile([128, 2, NQ], F32, tag="x_f32")
        layernorm(u1, vecs["n1_g"], vecs["n1_b"], x1_bf, x1_f32)

        # ================= spatial cross attention =================
        imgT = big.tile([128, 2, NK], BF16, tag="kvT")
        nc.sync.dma_start(out=imgT, in_=dr["imgT"].rearrange(
            "(kt p) n -> p kt n", p=128))
        O2u = big.tile([128, 2, NQ], BF16, tag="Ou")
        d_r2 = big.tile([128, 2, NQ], F32, tag="d_a2")
        attention(x1_bf, imgT, wts["s_wq"], wts["s_wk"], wts["s_wv"],
                  1.0 / (float(D) ** 0.5), O2u, d_r2)
        dbc2 = make_dbc(d_r2)
        # cross^T = s_wo^T @ O2u (unnormalized), then * dbc2 + s_bo + x1
        u2 = big.tile([128, 2, NQ], F32, tag="u")
        for et in range(2):
            for (off, cs) in C2:
                ps = pp.tile([128, 512], F32, tag="proj")
                for kt in range(2):
                    nc.tensor.matmul(
                        ps[:, :cs],
                        lhsT=wts["s_wo"][:, kt, 128 * et:128 * et + 128],
                        rhs=O2u[:, kt, off:off + cs],
                        start=(kt == 0), stop=(kt == 1))
                tt = big.tile([128, 512], F32, tag="res_t2")
                nc.vector.tensor_tensor(out=tt[:, :cs], in0=ps[:, :cs],
                                        in1=dbc2[:, et, off:off + cs],
                                        op=ALU.mult)
                nc.vector.tensor_scalar(out=tt[:, :cs], in0=tt[:, :cs],
                                        scalar1=vecs["s_bo"][:, et:et + 1],
                                        scalar2=None, op0=ALU.add)
                nc.vector.tensor_tensor(out=u2[:, et, off:off + cs],
                                        in0=tt[:, :cs],
                                        in1=x1_f32[:, et, off:off + cs],
                                        op=ALU.add)
        x2_bf = big.tile([128, 2, NQ], BF16, tag="x2_bf")
        x2_f32 = big.tile([128, 2, NQ], F32, tag="x_f32")
        layernorm(u2, vecs["n2_g"], vecs["n2_b"], x2_bf, x2_f32)

        # ================= FFN =================
        hT = big.tile([128, 4, NQ], BF16, tag="hT")
        for ft in range(4):
            for (off, cs) in C2:
                ps = pp.tile([128, 512], F32, tag="proj")
                for kt in range(2):
                    nc.tensor.matmul(
                        ps[:, :cs],
                        lhsT=ffw1[:, kt, 128 * ft:128 * ft + 128],
                        rhs=x2_bf[:, kt, off:off + cs],
                        start=(kt == 0), stop=(kt == 1))
                nc.vector.tensor_scalar(out=hT[:, ft, off:off + cs],
                                        in0=ps[:, :cs],
                                        scalar1=vecs["ff_b1"][:, ft:ft + 1],
                                        scalar2=0.0, op0=ALU.add,
                                        op1=ALU.max)
        u3 = big.tile([128, 2, NQ], F32, tag="u")
        for et in range(2):
            for (off, cs) in C2:
                ps = pp.tile([128, 512], F32, tag="proj")
                for kt in range(4):
                    nc.tensor.matmul(
                        ps[:, :cs],
                        lhsT=ffw2[:, kt, 128 * et:128 * et + 128],
                        rhs=hT[:, kt, off:off + cs],
                        start=(kt == 0), stop=(kt == 3))
                tt = big.tile([128, 512], F32, tag="res_t2")
                nc.vector.tensor_scalar(out=tt[:, :cs], in0=ps[:, :cs],
                                        scalar1=vecs["ff_b2"][:, et:et + 1],
                                        scalar2=None, op0=ALU.add)
                nc.vector.tensor_tensor(out=u3[:, et, off:off + cs],
                                        in0=tt[:, :cs],
                                        in1=x2_f32[:, et, off:off + cs],
                                        op=ALU.add)
        outT = big.tile([128, 2, NQ], F32, tag="qf32_out")
        layernorm(u3, vecs["n3_g"], vecs["n3_b"], None, outT)
        nc.sync.dma_start(out=out_d.rearrange("(kt p) n -> p kt n", p=128),
                          in_=outT)

    nc.compile()
    return nc


def _prep_inputs(inputs):
    bf = ml_dtypes.bfloat16
    f32 = np.float32

    def c(x, dt):
        return np.ascontiguousarray(np.asarray(x), dtype=dt)

    shared = {}
    for w in ("t_wq", "t_wk", "t_wv", "s_wq", "s_wk", "s_wv", "s_wo",
              "ff_w1", "ff_w2"):
        shared[w] = c(inputs[w], bf)
    for v, t in (("s_bo", 2), ("ff_b1", 4), ("ff_b2", 2), ("n1_g", 2),
                 ("n1_b", 2), ("n2_g", 2), ("n2_b", 2), ("n3_g", 2),
                 ("n3_b", 2)):
        shared[v] = c(np.asarray(inputs[v]).reshape(t, 128), f32)

    prevT = [c(np.asarray(inputs["prev_seq"][b]).T, bf) for b in range(B)]
    imgT = [c(np.asarray(inputs["img_ft"][b]).T, bf) for b in range(B)]

    in_maps = []
    for core in range(8):
        b, ch = core // 4, core % 4
        qc = np.asarray(inputs["query"][b, NQ * ch:NQ * (ch + 1), :]).T
        m = dict(shared)
        m["qT_bf"] = c(qc, bf)
        m["qT_f32"] = c(qc, f32)
        m["prevT"] = prevT[b]
        m["imgT"] = imgT[b]
        in_maps.append(m)
    return in_maps


def kernel(**inputs):
    from concourse.bass_utils import run_bass_kernel_spmd

    if "nc" not in _CACHE:
        _CACHE["nc"] = _build_nc()
    nc = _CACHE["nc"]

    in_maps = _prep_inputs(inputs)
    res = run_bass_kernel_spmd(nc, in_maps, core_ids=list(range(8)))

    out = np.empty((B, Q, E), np.float32)
    for core in range(8):
        b, ch = core // 4, core % 4
        out[b, NQ * ch:NQ * (ch + 1), :] = res.results[core]["outT"].T
    return out
